# revision 1
# baseline (speedup 1.0000x reference)
"""Trainium2 Bass kernel for nn_MeshPoolBlock (retrieval_knn).

Computes, for each of M=10000 query points (sub_vertices), the index of the
nearest of N=50000 vertices (squared-L2 argmin), then gathers the matching
rows of X [N, 256].

Strategy (8 NeuronCores, no collectives):
  - Shard the M dimension across the 8 cores (1250 queries each, padded to
    1280 = 128 partitions x 10 M-tiles).  Each core sees all N vertices.
  - argmin_n d2(n, m) == argmax_n score(n, m) with
        score = 2*v.s - |v|^2          (|s|^2 dropped: constant per query)
    computed as one K=4 augmented GEMM on the TensorEngine:
        lhsT (stationary) = [2sx, 2sy, 2sz, -1]   [4, 128]  per M-tile
        rhs  (moving)     = [vx,  vy,  vz, |v|^2] [4, 512]  per chunk
    The K=4 matmuls are packed 4x into the PE array via tile_position row
    groups (rows 0/32/64/96), one PSUM bank per group.
  - The DVE scans each PSUM block directly (1 elem/cycle/lane -- the fp32
    floor; PSUM has a single read port and the reduce uop has no 2x mode)
    using tensor_scalar with a fused op1=max accumulator, producing one
    running max per 2048-vertex superblock -> bm[128, 25].
  - Per M-tile the winning superblock per partition is found with max /
    max_index; its 2048 vertices are re-fetched per-partition via indirect
    DMA and the scores recomputed on the DVE to locate the exact argmax.
  - Final gather: indirect DMA of X rows by the winning indices.
"""

import numpy as np

import bass_rust
import concourse.bass as bass
import concourse.tile as tile
import concourse.mybir as mybir
from concourse import bass_utils

P = 128          # partitions
N = 50000        # vertices
NPAD = 51200     # padded to 25 superblocks of 2048
NSB = 25         # superblocks per M-tile scan
SB = 2048        # superblock (TTR block) size
M = 10000        # sub_vertices
F = 256          # feature dim of X
NCORES = 8
MC = M // NCORES          # 1250 queries per core
MCP = 1280                # padded to 128 * 10
T = MCP // P              # 10 M-tiles per core
BIG_NEG = -3.0e38

_f32 = mybir.dt.float32
_u32 = mybir.dt.uint32


def _build_program():
    nc = bass.Bass("TRN2", target_bir_lowering=False, debug=False)

    # c16 row 4g+k = group g, aug-coord k: [moving operand | stationary]
    c16 = nc.dram_tensor("c16", [16, NSB * 512 + T * P], _f32, kind="ExternalInput")
    a_super = nc.dram_tensor("a_super", [NPAD // 128, 128 * 4], _f32, kind="ExternalInput")
    b_lanes = nc.dram_tensor("b_lanes", [P, T * 4], _f32, kind="ExternalInput")
    x_in = nc.dram_tensor("x_in", [N, F], _f32, kind="ExternalInput")
    out = nc.dram_tensor("out", [MCP, F], _f32, kind="ExternalOutput")
    idx_out = nc.dram_tensor("idx_out", [P, T], _u32, kind="ExternalOutput")

    mx = mybir.AluOpType.max

    with tile.TileContext(nc) as tc:
        with (
            tc.tile_pool(name="const", bufs=1) as constp,
            tc.tile_pool(name="psum", bufs=2, space="PSUM") as psump,
            tc.tile_pool(name="sblk", bufs=3) as sblkp,
            tc.tile_pool(name="l1", bufs=2) as l1p,
            tc.tile_pool(name="winv", bufs=2) as winp,
            tc.tile_pool(name="sw", bufs=2) as swp,
            tc.tile_pool(name="small", bufs=4) as smallp,
            tc.tile_pool(name="persist", bufs=1) as perp,
        ):
            # one DMA scatters c16 rows 4g+k onto SBUF partitions 32g+k
            cst = constp.tile([P, NSB * 512 + T * P], _f32)
            wl = constp.tile([P, T * 4], _f32)
            for g in range(4):
                nc.sync.dma_start(
                    out=cst[32 * g:32 * g + 4, :], in_=c16[4 * g:4 * g + 4, :]
                )
            nc.sync.dma_start(out=wl[:], in_=b_lanes[:])
            AMOV0 = 0                  # moving operand base in cst
            BST0 = NSB * 512           # stationary base in cst

            idxall = perp.tile([P, T], _u32)
            xbuf = perp.tile([P, T * F], _f32)

            GRAN = 128                     # granule: finest max granularity
            NGR = NPAD // GRAN             # 400 granules per M-tile
            GPB = SB // GRAN               # 16 granules per scan block
            for t in range(T):
                bmall = l1p.tile([P, NGR], _f32, tag="bmall")
                for b in range(NSB):
                    ps = psump.tile([P, SB], _f32)
                    for g in range(4):
                        nc.tensor.matmul(
                            out=ps[:, 512 * g:512 * (g + 1)],
                            lhsT=cst[32 * g:32 * g + 4, BST0 + P * t:BST0 + P * (t + 1)],
                            rhs=cst[32 * g:32 * g + 4, AMOV0 + 512 * b:AMOV0 + 512 * (b + 1)],
                            start=True,
                            stop=True,
                            tile_position=(32 * g, 0),
                        )
                    # ScalarE evacuates the block (frees PSUM for the PE
                    # sooner); the DVE granule-reduce then reads SBUF, which
                    # is ~120 cycles/op cheaper than a PSUM source.
                    sblk = sblkp.tile([P, SB], _f32)
                    nc.scalar.copy(sblk[:], ps[:])
                    sb3 = sblk[:].rearrange("p (k m) -> p k m", m=GRAN)
                    nc.vector.tensor_reduce(
                        out=bmall[:, GPB * b:GPB * (b + 1)],
                        in_=sb3,
                        axis=mybir.AxisListType.X,
                        op=mx,
                    )

                # winning granule per partition
                top8 = smallp.tile([P, 8], _f32, tag="top8")
                nc.vector.max(top8[:], bmall[:])
                b8 = smallp.tile([P, 8], _u32, tag="b8")
                nc.vector.max_index(b8[:], top8[:], bmall[:])

                # per-partition fetch of the winning granule's vertex rows
                winv = winp.tile([P, GRAN * 4], _f32)
                nc.gpsimd.indirect_dma_start(
                    out=winv[:],
                    out_offset=None,
                    in_=a_super[:],
                    in_offset=bass.IndirectOffsetOnAxis(ap=b8[:, 0:1], axis=0),
                )

                # recompute scores for the window:  sum_k w[k] * vert[k]
                wv = winv[:].rearrange("p (n k) -> p n k", k=4)
                sw = swp.tile([P, GRAN], _f32)
                nc.vector.tensor_scalar(
                    out=sw[:],
                    in0=wv[:, :, 0],
                    scalar1=wl[:, 4 * t + 0:4 * t + 1],
                    scalar2=None,
                    op0=mybir.AluOpType.mult,
                )
                for k in (1, 2, 3):
                    nc.vector.scalar_tensor_tensor(
                        out=sw[:],
                        in0=wv[:, :, k],
                        scalar=wl[:, 4 * t + k:4 * t + k + 1],
                        in1=sw[:],
                        op0=mybir.AluOpType.mult,
                        op1=mybir.AluOpType.add,
                    )

                w8 = smallp.tile([P, 8], _f32, tag="w8")
                nc.vector.max(w8[:], sw[:])
                j8 = smallp.tile([P, 8], _u32, tag="j8")
                nc.vector.max_index(j8[:], w8[:], sw[:])

                # n = b*2048 + j   (exact fp32 integer math), clamp, cast
                bf = smallp.tile([P, 1], _f32, tag="bf")
                nc.vector.tensor_copy(bf[:], b8[:, 0:1])
                jf = smallp.tile([P, 1], _f32, tag="jf")
                nc.vector.tensor_copy(jf[:], j8[:, 0:1])
                nf = smallp.tile([P, 1], _f32, tag="nf")
                nc.vector.scalar_tensor_tensor(
                    out=nf[:],
                    in0=bf[:],
                    scalar=float(GRAN),
                    in1=jf[:],
                    op0=mybir.AluOpType.mult,
                    op1=mybir.AluOpType.add,
                )
                nc.vector.tensor_scalar_min(nf[:], nf[:], float(N - 1))
                nc.vector.tensor_copy(idxall[:, t:t + 1], nf[:])

                # gather this tile's X rows immediately (overlaps later tiles)
                nc.gpsimd.indirect_dma_start(
                    out=xbuf[:, F * t:F * (t + 1)],
                    out_offset=None,
                    in_=x_in[:],
                    in_offset=bass.IndirectOffsetOnAxis(ap=idxall[:, t:t + 1], axis=0),
                )

            # out row (p*T + t) = xbuf[p, t*F:(t+1)*F]
            out_v = out.ap().rearrange("(p t) f -> p (t f)", p=P)
            nc.sync.dma_start(out=out_v, in_=xbuf[:])
            nc.sync.dma_start(out=idx_out[:], in_=idxall[:])

    # TRN2 hardware allows at most one sync wait per instruction (the
    # LDWEIGHTS half of a self-loading fp32 matmul in particular); split
    # multi-wait instructions into event-semaphore chains.
    bass_rust.generate_event_semaphores(nc)
    return nc


def _prep_host(vertices, sub_vertices, X):
    vertices = np.ascontiguousarray(vertices, dtype=np.float32)
    sub_vertices = np.ascontiguousarray(sub_vertices, dtype=np.float32)
    X = np.ascontiguousarray(X, dtype=np.float32)

    v2 = (vertices * vertices).sum(axis=1, dtype=np.float32)
    a_aug = np.zeros((4, NPAD), dtype=np.float32)
    a_aug[0:3, :N] = vertices.T
    a_aug[3, :N] = v2
    a_aug[3, N:] = 1.0e30  # padding: score = -1e30, never wins

    # moving operand layout: group g holds vertices n = 2048*b + 512*g + j
    a_sb = (
        a_aug.reshape(4, NSB, 4, 512).transpose(2, 0, 1, 3).reshape(16, NSB * 512)
    )
    a_sb = np.ascontiguousarray(a_sb)
    # row-interleaved copy for the per-partition window re-fetch
    a_super = np.ascontiguousarray(a_aug.T.reshape(NPAD // 128, 128 * 4))

    per_core = []
    for c in range(NCORES):
        sub = sub_vertices[c * MC:(c + 1) * MC]
        subp = np.concatenate([sub, np.broadcast_to(sub[0], (MCP - MC, 3))], axis=0)
        # lane (p, t) serves local query m = p*T + t
        m_of = (np.arange(P)[:, None] * T + np.arange(T)[None, :])  # [P, T]
        w = np.empty((P, T, 4), dtype=np.float32)
        w[:, :, 0:3] = 2.0 * subp[m_of]
        w[:, :, 3] = -1.0
        ws = w.transpose(2, 1, 0).reshape(4, T * P)  # [k, t*P + p]
        b_stat = np.tile(ws[None], (4, 1, 1)).reshape(16, T * P)
        c16 = np.ascontiguousarray(np.concatenate([a_sb, b_stat], axis=1))
        b_lanes = np.ascontiguousarray(w.reshape(P, T * 4))
        per_core.append((c16, b_lanes))
    return a_super, X, per_core


TRACE = False
LAST_RESULTS = None


def kernel(vertices, sub_vertices, X):
    global LAST_RESULTS
    in_dtype = np.asarray(X).dtype
    a_super, Xc, per_core = _prep_host(
        np.asarray(vertices), np.asarray(sub_vertices), np.asarray(X)
    )
    nc = _build_program()
    in_maps = []
    for c in range(NCORES):
        c16, b_lanes = per_core[c]
        in_maps.append(
            {
                "c16": c16,
                "a_super": a_super,
                "b_lanes": b_lanes,
                "x_in": Xc,
            }
        )
    res = bass_utils.run_bass_kernel_spmd(
        nc, in_maps, core_ids=list(range(NCORES)), trace=TRACE
    )
    LAST_RESULTS = res
    outs = [np.asarray(res.results[c]["out"])[:MC] for c in range(NCORES)]
    return np.concatenate(outs, axis=0).astype(in_dtype, copy=False)

